# revision 1
# baseline (speedup 1.0000x reference)
"""Trainium2 Bass kernel for BottleneckAttention (patch attention).

q patches [160, 5120] from z1_hat (non-overlapping 10x4 unfold),
kv patches [5551, 5120] from z2 (overlapping unfold, Hk=91 x Wk=61),
scores = q @ kv.T / 5120, softmax over kv patches, out = attn @ kv,
folded back to [1, 128, 100, 64].

Sharding: contiguous blocks of 12 kv h-rows per core (8 x 12 = 96 >= 91).
Each core owns the 768 flat positions p = h_local*64 + w (w in [0,64);
positions with w >= 61 or h >= 91 are invalid -- their kv rows are zeroed
so they never touch the numerator, and the host subtracts their exactly
recomputed exp contribution from the denominator. Every core computes all
160 q rows; the host combines with an all-gather softmax.

Per-core kernel (raw Bass, explicit semaphores):
  phase 1 (bf16): scores as implicit convolution against the SBUF-resident
    z2 slice, streamed as CONTIGUOUS 448/320-column flat windows from 4
    byte-shifted copies (one per kernel column offset j), emitted as one
    long PSUM accumulation chain per score tile so the PE pipelines
    back-to-back matmuls. The w>=61 junk columns land on zeroed kv rows
    and are subtracted from the denominator on the host.
  exp on ScalarE (scale = 1/5120), row-sum denominator on VectorE.
  PE transpose of exp-scores; the PSUM->SBUF copy applies bias=-1 so the
  bf16 e_T actually stores f = e-1 (centered softmax: |f| <~ 0.08 keeps
  absolute precision; the host adds the exact sum-of-kv-columns term).
  phase 2 (bf16): partial_out = f_T.T @ kv_shard, kv resident in SBUF.
"""

import sys

sys.path.insert(0, "/opt/trn_rl_repo")

import numpy as np
import ml_dtypes

import concourse.bass as bass
import concourse.mybir as mybir

DT = mybir.dt
AF = mybir.ActivationFunctionType

# problem geometry (hardcoded from the reference module)
KC, KH, KW = 128, 10, 4
H, W = 100, 64
NH, NW = H // KH, W // KW          # 10, 16
PQ = NH * NW                       # 160 q patches
D = KC * KH * KW                   # 5120
HK, WK = H - KH + 1, W - KW + 1    # 91, 61
NCORES = 8
HPC = 12                           # kv h-rows per core (8*12 = 96 >= 91)
PKC = HPC * W                      # 768 flat positions per core
T = 6                              # 768 / 128 k-chunks for phase 2
G0H, G1H = 7, 5                    # phase-1 h-groups (7+5 = 12)
N0 = G0H * W                       # 448: contiguous stream for h 0..6
N1 = G1H * W                       # 320: contiguous stream for h 7..11
OFF1 = G0H * W                     # 448: flat offset of group 1
ZROWS = 2 * HPC                    # 24 z rows staged per core
NT = D // 512                      # 10 phase-2 n-tiles
SCALE = 1.0 / D

P1_NP = ml_dtypes.bfloat16

_CACHE = {}


def _build_nc():
    nc = bass.Bass()
    z_d = nc.declare_dram_parameter("z", [KC, ZROWS * W], DT.bfloat16, isOutput=False)
    q_d = nc.declare_dram_parameter("qT3", [KC, KH * KW, PQ], DT.bfloat16, isOutput=False)
    kv_d = nc.declare_dram_parameter("kvr", [128, T, D], DT.bfloat16, isOutput=False)
    out_d = nc.declare_dram_parameter("out", [PQ, D], DT.float32, isOutput=True)
    den_d = nc.declare_dram_parameter("den", [PQ + 32, 1], DT.float32, isOutput=True)

    from contextlib import ExitStack

    ctx = ExitStack()
    with ctx:
        # 4 byte-shifted copies of flat z so every (i,j) stream is 128B-aligned
        z_sb = ctx.enter_context(nc.sbuf_tensor([KC, KW, ZROWS * W], DT.bfloat16))
        q_sb = ctx.enter_context(nc.sbuf_tensor([KC, KH * KW, PQ], DT.bfloat16))
        kv_sb = ctx.enter_context(nc.sbuf_tensor([128, T, D], DT.bfloat16))
        e_hi = ctx.enter_context(nc.sbuf_tensor([128, PKC], DT.float32))
        e_lo = ctx.enter_context(nc.sbuf_tensor([64, PKC], DT.float32))
        eT_sb = ctx.enter_context(nc.sbuf_tensor([128, T, PQ], DT.bfloat16))
        o_hi = ctx.enter_context(nc.sbuf_tensor([128, D], DT.float32))
        o_lo = ctx.enter_context(nc.sbuf_tensor([64, NT // 2, 512], DT.float32))
        iden = ctx.enter_context(nc.sbuf_tensor([128, 128], DT.float32))
        wz = ctx.enter_context(nc.sbuf_tensor([128, 512], DT.bfloat16))
        bias0 = ctx.enter_context(nc.sbuf_tensor([128, 1], DT.float32))
        dh_sb = ctx.enter_context(nc.sbuf_tensor([128, 1], DT.float32))
        dl_sb = ctx.enter_context(nc.sbuf_tensor([64, 1], DT.float32))

        # phase-1 score accumulators: (h-group, q-half)
        ps_s = [
            ctx.enter_context(nc.psum_tensor(f"ps_s{i}", [128, n], DT.float32))
            for i, n in enumerate((N0, N0, N1, 384))
        ]  # order: g0m0, g0m1, g1m0, g1m1
        # transpose staging / phase-2 accumulators (4 distinct banks)
        ps_t = [
            ctx.enter_context(nc.psum_tensor(f"ps_t{i}", [128, 512], DT.float32))
            for i in range(4)
        ]

        s_z = ctx.enter_context(nc.semaphore("s_z"))
        s_qq = [ctx.enter_context(nc.semaphore(f"s_qq{i}")) for i in range(4)]
        s_kv = ctx.enter_context(nc.semaphore("s_kv"))
        s_p = ctx.enter_context(nc.semaphore("s_p"))
        s_a = ctx.enter_context(nc.semaphore("s_a"))
        s_v = ctx.enter_context(nc.semaphore("s_v"))
        s_g = ctx.enter_context(nc.semaphore("s_g"))
        s_zs = ctx.enter_context(nc.semaphore("s_zs"))
        s_o = ctx.enter_context(nc.semaphore("s_o"))

        # transposes: all 6 m0 chunks (run between the m0 and m1 score
        # chains, keeping the PE busy so HAM stays warm), then the 6 m1
        # chunks after the m1 chains.
        # ACT order / s_a values: exp g0m0=1, exp g1m0=2, m0 copies 3..8,
        # exp g0m1=9, exp g1m1=10, m1 copies 11..16, out-copies 17..36.
        tr_m0 = [(t, 0, 2) for t in range(6)]
        # m1 transposes: (e_lo rows, e_lo col range, iden base, psum rows,
        # eT chunk, eT row range, s_a threshold). The paired m1 chains put
        # the g0 scores at partitions 0-31 and g1 at 32-63, so chunk 3
        # (cols 384..512) splits into two pieces.
        # m1 chains split 384/384 so every transpose chunk is a full,
        # partition-0-aligned 128 columns (transpose psum must start at 0)
        TRM1 = [(t, 9 if t < 3 else 10) for t in range(6)]

        with nc.Block() as block:

            @block.gpsimd
            def _(g):
                g.memset(wz[:], 0.0).then_inc(s_g, 1)        # 1: warmup tile
                g.memset(iden[:], 0.0)
                g.affine_select(
                    out=iden[:],
                    in_=iden[:],
                    compare_op=mybir.AluOpType.not_equal,
                    fill=1.0,
                    base=0,
                    pattern=[[-1, 128]],
                    channel_multiplier=1,
                ).then_inc(s_g, 1)                            # 2: identity
                g.memset(bias0[:], 0.0).then_inc(s_g, 1)      # 3: bias

            @block.sync
            def _(sync):
                sync.dma_start(z_sb[:, 0, :], z_d[:]).then_inc(s_z, 16)
                # q in quarters, each with its own semaphore (completion
                # order across DMA queues is not guaranteed)
                for qtr in range(4):
                    sl = slice(10 * qtr, 10 * qtr + 10)
                    sync.dma_start(q_sb[:, sl, :], q_d[:, sl, :]).then_inc(
                        s_qq[qtr], 16
                    )
                for c in range(3):
                    sync.dma_start(
                        kv_sb[:, 2 * c : 2 * c + 2, :], kv_d[:, 2 * c : 2 * c + 2, :]
                    ).then_inc(s_kv, 16)
                sync.wait_ge(s_v, 1)
                sync.dma_start(den_d[0:128, :], dh_sb[:]).then_inc(s_o, 16)
                sync.wait_ge(s_v, 3)
                sync.dma_start(den_d[128:192, :], dl_sb[:]).then_inc(s_o, 16)
                # out halves pipelined behind the ACT psum->sbuf copies
                # (out-copy g bumps s_a to 17+g; m0 tiles are g 0..9)
                sync.wait_ge(s_a, 21)
                sync.dma_start(out_d[0:128, 0:2560], o_hi[:, 0:2560]).then_inc(s_o, 16)
                sync.wait_ge(s_a, 26)
                sync.dma_start(out_d[0:128, 2560:], o_hi[:, 2560:]).then_inc(s_o, 16)
                out_lo4 = out_d[128:160, :].rearrange(
                    "p (a b c) -> p a b c", a=NT // 2, b=2, c=512
                )
                # pair j's copies land at s_a = 27+2j (even cols) / 28+2j
                sync.wait_ge(s_a, 31)
                sync.dma_start(out_lo4[:, 0:3, 0, :], o_lo[0:32, 0:3, :]).then_inc(
                    s_o, 16
                )
                sync.wait_ge(s_a, 32)
                sync.dma_start(out_lo4[:, 0:3, 1, :], o_lo[32:64, 0:3, :]).then_inc(
                    s_o, 16
                )
                sync.wait_ge(s_a, 35)
                sync.dma_start(out_lo4[:, 3:5, 0, :], o_lo[0:32, 3:5, :]).then_inc(
                    s_o, 16
                )
                sync.wait_ge(s_a, 36)
                sync.dma_start(out_lo4[:, 3:5, 1, :], o_lo[32:64, 3:5, :]).then_inc(
                    s_o, 16
                )
                sync.wait_ge(s_o, 128)

            @block.tensor
            def _(pe):
                # HAM warmup on the zeroed bf16 tile while input DMAs land:
                # phase 1 then starts at the warm 2.4 GHz clock.
                pe.wait_ge(s_g, 1)
                for _w in range(9):
                    nc.tensor.matmul(
                        ps_t[0][0:128, 0:512],
                        wz[:, 0:128],
                        wz[:, 0:512],
                        start=(_w == 0),
                        stop=(_w == 8),
                    )
                pe.wait_ge(s_z, 16)
                pe.wait_ge(s_zs, 3)
                pe.wait_ge(s_qq[0], 16)
                # phase 1: scores[pq, pos] += q(:,ij,:).T @ zflat[:, off+pos]
                # contiguous streams; junk cols (w>=61) corrected on host.
                # One long accumulation chain per psum group -- the PE only
                # pipelines back-to-back matmuls within a group.
                def chain(grp, m, first):
                    ps = ps_s[grp * 2 + m]
                    dst = ps[:, :] if m == 0 else ps[0:32, :]
                    msl = slice(0, 128) if m == 0 else slice(128, 160)
                    for ij in range(KH * KW):
                        if first and ij in (10, 20, 30):
                            pe.wait_ge(s_qq[ij // 10], 16)
                        i_, j_ = ij // KW, ij % KW
                        st, sp = ij == 0, ij == KH * KW - 1
                        base = i_ * W + (OFF1 if grp == 1 else 0)
                        rhs = z_sb[:, j_, base : base + (N1 if grp == 1 else N0)]
                        mm = nc.tensor.matmul(
                            dst, q_sb[:, ij, msl], rhs, start=st, stop=sp
                        )
                    return mm

                def transposes(batch, k0, frees):
                    for k, (t, m, thr) in enumerate(batch, start=k0):
                        msz = 128 if m == 0 else 32
                        esrc = (
                            e_hi[:, t * 128 : (t + 1) * 128]
                            if m == 0
                            else e_lo[:, t * 128 : (t + 1) * 128]
                        )
                        # ps_t[k%4] free once ACT copy k-4 ran
                        freed = 0 if k < 4 else frees[k - 4]
                        pe.wait_ge(s_a, max(thr, freed))
                        nc.tensor.matmul(
                            ps_t[k % 4][0:128, 0:msz],
                            esrc,
                            iden[0:msz, 0:msz],
                            is_transpose=True,
                            start=True,
                            stop=True,
                        ).then_inc(s_p, 1)

                # ACT copy k lands at s_a: m0 k0-5 -> 3..8, m1 k6-12 ->
                # 11..17 (exps at 1, 2, 9, 10)
                COPY_SA = [3, 4, 5, 6, 7, 8, 11, 12, 13, 14, 15, 16]
                chain(0, 0, True).then_inc(s_p, 1)   # s_p = 1
                chain(1, 0, False).then_inc(s_p, 1)  # s_p = 2
                pe.wait_ge(s_g, 2)
                transposes(tr_m0, 0, COPY_SA)        # s_p = 3..8
                # m1 score chains: g0 and g1 run concurrently in disjoint
                # 32-wide PE column groups (psum bases 0 and 32)
                for ij in range(KH * KW):
                    i_, j_ = ij // KW, ij % KW
                    st, sp_ = ij == 0, ij == KH * KW - 1
                    mmA = nc.tensor.matmul(
                        ps_s[1][0:32, 0:384],
                        q_sb[:, ij, 128:160],
                        z_sb[:, j_, i_ * W : i_ * W + 384],
                        start=st,
                        stop=sp_,
                    )
                    mmB = nc.tensor.matmul(
                        ps_s[3][32:64, 0:384],
                        q_sb[:, ij, 128:160],
                        z_sb[:, j_, i_ * W + 384 : i_ * W + 768],
                        start=st,
                        stop=sp_,
                    )
                mmA.then_inc(s_p, 1)  # s_p = 9
                mmB.then_inc(s_p, 1)  # s_p = 10
                for k, (t, thr) in enumerate(TRM1, start=6):
                    freed = COPY_SA[k - 4]
                    pe.wait_ge(s_a, max(thr, freed))
                    rsl = slice(0, 32) if t < 3 else slice(32, 64)
                    ib = 0 if t < 3 else 32
                    nc.tensor.matmul(
                        ps_t[k % 4][0:128, 0:32],
                        e_lo[rsl, t * 128 : (t + 1) * 128],
                        iden[ib : ib + 32, ib : ib + 32],
                        is_transpose=True,
                        start=True,
                        stop=True,
                    ).then_inc(s_p, 1)  # s_p = 11..16
                # phase 2: out[pq, d] = sum_t fT[., t, pq].T @ kv[., t, d]
                pe.wait_ge(s_a, 16)
                pe.wait_ge(s_kv, 48)
                for gidx in range(NT):
                    if gidx >= 4:
                        pe.wait_ge(s_a, 13 + gidx)  # out-copy gidx-4 done
                    for t in range(T):
                        mm = nc.tensor.matmul(
                            ps_t[gidx % 4][0:128, 0:512],
                            eT_sb[:, t, 0:128],
                            kv_sb[:, t, gidx * 512 : (gidx + 1) * 512],
                            start=(t == 0),
                            stop=(t == T - 1),
                        )
                    mm.then_inc(s_p, 1)  # s_p = 17+gidx
                # q-rows 128..159 (M=32): pairs of n-tiles run concurrently
                # in disjoint 32-wide PE column groups (tile_position derives
                # from the psum base partition: 0 vs 32)
                for j in range(NT // 2):
                    gA, gB = 10 + 2 * j, 11 + 2 * j
                    pe.wait_ge(s_a, 13 + gA)
                    pe.wait_ge(s_a, 13 + gB)
                    bA, bB = ps_t[gA % 4], ps_t[gB % 4]
                    for t in range(T):
                        mmA = nc.tensor.matmul(
                            bA[0:32, 0:512],
                            eT_sb[:, t, 128:160],
                            kv_sb[:, t, (2 * j) * 512 : (2 * j + 1) * 512],
                            start=(t == 0),
                            stop=(t == T - 1),
                        )
                        mmB = nc.tensor.matmul(
                            bB[32:64, 0:512],
                            eT_sb[:, t, 128:160],
                            kv_sb[:, t, (2 * j + 1) * 512 : (2 * j + 2) * 512],
                            start=(t == 0),
                            stop=(t == T - 1),
                        )
                    mmA.then_inc(s_p, 1)  # s_p = 27+2j
                    mmB.then_inc(s_p, 1)  # s_p = 28+2j

            @block.scalar
            def _(act):
                def expcall(ps, esl, b):
                    nc.scalar.activation(
                        esl, ps, AF.Exp, bias=b, scale=SCALE
                    ).then_inc(s_a, 1)

                def trcopy(batch, k0, poff):
                    # transposed chunks -> f_T = e - 1 (cast to bf16)
                    for k, (t, m, _thr) in enumerate(batch, start=k0):
                        m0, msz = (0, 128) if m == 0 else (128, 32)
                        act.wait_ge(s_p, poff + k)
                        nc.scalar.activation(
                            eT_sb[:, t, m0 : m0 + msz],
                            ps_t[k % 4][0:128, 0:msz],
                            AF.Copy,
                            bias=-1.0,
                        ).then_inc(s_a, 1)

                act.wait_ge(s_g, 3)
                act.wait_ge(s_p, 1)
                expcall(ps_s[0][:, :], e_hi[:, 0:N0], bias0[:, :])        # s_a=1
                act.wait_ge(s_p, 2)
                expcall(ps_s[2][:, :], e_hi[:, OFF1 : OFF1 + N1], bias0[:, :])  # 2
                trcopy(tr_m0, 0, 3)                                      # s_a=3..8
                act.wait_ge(s_p, 9)
                expcall(ps_s[1][0:32, 0:384], e_lo[0:32, 0:384], bias0[0:32, :])  # 9
                act.wait_ge(s_p, 10)
                expcall(
                    ps_s[3][32:64, 0:384],
                    e_lo[32:64, 384:768],
                    bias0[32:64, :],
                )  # s_a=10
                for k, (t, thr) in enumerate(TRM1, start=6):
                    act.wait_ge(s_p, 5 + k)
                    nc.scalar.activation(
                        eT_sb[:, t, 128:160],
                        ps_t[k % 4][0:128, 0:32],
                        AF.Copy,
                        bias=-1.0,
                    ).then_inc(s_a, 1)  # s_a = 11..16
                # copy phase-2 accumulators to out staging
                for gidx in range(NT):
                    act.wait_ge(s_p, 17 + gidx)
                    nc.scalar.activation(
                        o_hi[:, gidx * 512 : (gidx + 1) * 512],
                        ps_t[gidx % 4][0:128, 0:512],
                        AF.Copy,
                    ).then_inc(s_a, 1)  # s_a = 17+gidx
                for j in range(NT // 2):
                    act.wait_ge(s_p, 27 + 2 * j)
                    nc.scalar.activation(
                        o_lo[0:32, j, :], ps_t[(10 + 2 * j) % 4][0:32, 0:512], AF.Copy
                    ).then_inc(s_a, 1)  # s_a = 27+2j
                    act.wait_ge(s_p, 28 + 2 * j)
                    nc.scalar.activation(
                        o_lo[32:64, j, :],
                        ps_t[(11 + 2 * j) % 4][32:64, 0:512],
                        AF.Copy,
                    ).then_inc(s_a, 1)  # s_a = 28+2j

            @block.vector
            def _(dve):
                # build the 3 byte-shifted z slabs on-chip (saves DMA bytes)
                dve.wait_ge(s_z, 16)
                for s in range(1, KW):
                    nc.vector.tensor_copy(
                        z_sb[:, s, 0 : ZROWS * W - s], z_sb[:, 0, s:]
                    ).then_inc(s_zs, 1)
                dve.wait_ge(s_a, 2)
                nc.vector.reduce_sum(
                    dh_sb[:], e_hi[:, :], axis=mybir.AxisListType.X
                ).then_inc(s_v, 1)
                dve.wait_ge(s_a, 9)
                nc.vector.reduce_sum(
                    dl_sb[0:32, :], e_lo[0:32, 0:384], axis=mybir.AxisListType.X
                ).then_inc(s_v, 1)
                dve.wait_ge(s_a, 10)
                nc.vector.reduce_sum(
                    dl_sb[32:64, :],
                    e_lo[32:64, 384:PKC],
                    axis=mybir.AxisListType.X,
                ).then_inc(s_v, 1)

    return nc


def _host_prep(z1_hat, z2):
    z1 = np.asarray(z1_hat, dtype=np.float32)[0]  # [128, 100, 64]
    z2a = np.asarray(z2, dtype=np.float32)[0]

    # q patches [160, 5120] and lhsT layout qT3 [128, 40, 160]
    q = z1.reshape(KC, NH, KH, NW, KW).transpose(1, 3, 0, 2, 4).reshape(PQ, D)
    qT3 = np.ascontiguousarray(
        q.reshape(PQ, KC, KH * KW).transpose(1, 2, 0).astype(P1_NP)
    )

    # padded z2: rows 100..111 zero
    z_pad = np.zeros((KC, 112, W), dtype=np.float32)
    z_pad[:, :H] = z2a

    # sliding kv patches from padded z2
    sw = np.lib.stride_tricks.sliding_window_view(z_pad, (KH, KW), axis=(1, 2))
    # sw: [128, 103, 61, 10, 4]; patch(h, w) = sw[:, h, w]

    q64 = q.astype(np.float64)
    ij_off = (np.arange(KH)[:, None] * W + np.arange(KW)[None, :]).reshape(-1)  # [40]

    in_maps = []
    corrs = []
    for core in range(NCORES):
        h0 = HPC * core
        zf = z_pad[:, h0 : h0 + ZROWS, :].reshape(KC, ZROWS * W)
        # kv rows indexed by flat position p = h_local*64 + w
        kvp = np.zeros((PKC, D), dtype=np.float32)
        hh = np.arange(PKC) // W
        ww = np.arange(PKC) % W
        real = (ww < WK) & (h0 + hh < HK)
        ridx = np.nonzero(real)[0]
        kvp[ridx] = (
            sw[:, h0 + hh[ridx], ww[ridx]].transpose(1, 0, 2, 3).reshape(-1, D)
        )
        kvr = np.ascontiguousarray(
            kvp.reshape(T, 128, D).transpose(1, 0, 2).astype(ml_dtypes.bfloat16)
        )
        in_maps.append(
            {
                "z": np.ascontiguousarray(zf.astype(P1_NP)),
                "qT3": qT3,
                "kvr": kvr,
            }
        )
        # denominator correction: computed-but-invalid columns. The device
        # computes exp(q . window / D) for every position in the two
        # contiguous streams [0,445) and [448,765); positions that are not
        # real patches (w >= 61 or h >= 91) polluted the on-chip row-sum.
        # streams now cover every flat position; invalid = not a real patch
        bad = np.nonzero(~real)[0]
        win = zf.astype(np.float64)[:, bad[:, None] + ij_off[None, :]]  # [128,nb,40]
        patches = win.transpose(1, 0, 2).reshape(len(bad), D)  # d-order (c, i, j)
        s_bad = q64 @ patches.T  # [160, nb]
        corrs.append(np.exp(s_bad * SCALE).sum(axis=1))

    corr = np.sum(corrs, axis=0)
    # centered softmax: device returns f @ kv with f = e - 1; host adds the
    # exact colsum term sum_k kv[k, :] over all real patches (all cores).
    swr = sw[:, :HK, :WK]
    colsum = swr.astype(np.float64).sum(axis=(1, 2)).reshape(D)  # [5120]
    return in_maps, corr, colsum


def kernel(z1_hat, z2):
    from concourse.bass_utils import run_bass_kernel_spmd

    in_maps, corr, colsum = _host_prep(z1_hat, z2)
    if "nc" not in _CACHE:
        _CACHE["nc"] = _build_nc()
    nc = _CACHE["nc"]
    res = run_bass_kernel_spmd(nc, in_maps, list(range(NCORES)))
    num = np.broadcast_to(colsum, (PQ, D)).astype(np.float64).copy()
    den = -corr
    for r in res.results:
        num += r["out"].astype(np.float64)
        dv = r["den"].astype(np.float64)[:, 0]
        den = den + np.concatenate([dv[0:128], dv[128:160] + dv[160:192]])
    out = (num / den[:, None]).astype(np.float32)
    # fold patches back: [160, 5120] -> [1, 128, 100, 64]
    out = out.reshape(NH, NW, KC, KH, KW).transpose(2, 0, 3, 1, 4)
    return np.ascontiguousarray(out.reshape(1, KC, H, W))



# revision 3
# speedup vs baseline: 1.6979x; 1.6979x over previous
"""Trainium2 Bass kernel for BottleneckAttention (patch attention).

q patches [160, 5120] from z1_hat (non-overlapping 10x4 unfold),
kv patches [5551, 5120] from z2 (overlapping unfold, Hk=91 x Wk=61),
scores = q @ kv.T / 5120, softmax over kv patches, out = attn @ kv,
folded back to [1, 128, 100, 64].

Sharding: contiguous blocks of 12 kv h-rows per core (8 x 12 = 96 >= 91).
Core owns 768 flat positions pos = h_local*64 + w (6 chunks of 128);
positions with w >= 61 or h >= 91 are invalid and their f values are
masked to zero on-chip, so neither numerator nor denominator needs any
host-side correction.

Fully transposed formulation (no PE transposes anywhere):
  phase 1: scoresT[pos, q] as implicit convolution: for each chunk of 128
    positions, accumulate over the 40 kernel offsets (i,j):
    psum[pos, q] += z4[:, j, i*64+t*128 :+128].T @ qT[:, ij, :]
    where z4 holds 4 byte-shifted copies of the core's flat z2 slab.
    fp8e4 DoubleRow: the (j, j+1) shifted copies form the two contraction
    planes of one matmul -> 20 matmuls per chunk, 2x PE throughput.
  ACT: e = exp(scores/5120) (psum -> fp32 sbuf).
  DVE: f = (e * mask) - mask -> fp8 eT  (centered softmax f = e-1 keeps
    fp8 absolute precision; mask zeroes invalid positions).
  den: ones.T @ f via a tiny 1-row matmul chain -> [1, 160].
  phase 2: implicit convolution out3[c, (i,j), q] += zT_shift.T @ f:
    for each (i,j), contract over the 6 position chunks; zt holds 8
    partition-shifted transposed copies of the z slab so every (i,j)
    offset is a clean chunk-aligned slice. fp8 DoubleRow pairs chunks
    (t, t+1). M=128 (channels), N=160 (queries): full PE utilization.

Host: adds the exact colsum term (sum of kv rows, via box filter) to the
f-numerator, 5551 to the f-denominator, divides, folds patches back.
"""

import sys

sys.path.insert(0, "/opt/trn_rl_repo")

import numpy as np
import ml_dtypes

import concourse.bass as bass
import concourse.mybir as mybir

DT = mybir.dt
AF = mybir.ActivationFunctionType
ALU = mybir.AluOpType
PM = mybir.MatmulPerfMode.DoubleRow

# problem geometry (hardcoded from the reference module)
KC, KH, KW = 128, 10, 4
H, W = 100, 64
NH, NW = H // KH, W // KW          # 10, 16
PQ = NH * NW                       # 160 q patches
D = KC * KH * KW                   # 5120
HK, WK = H - KH + 1, W - KW + 1    # 91, 61
NCORES = 8
HPC = 12                           # kv h-rows per core (8*12 = 96 >= 91)
PKC = HPC * W                      # 768 flat positions per core
T = PKC // 128                     # 6 position chunks
ZF = 1344                          # z4 flat length: (HPC + KH - 1 = 21) * 64
ZTC = 11                           # zt chunks: ceil((768 + 576 + 64) / 128)
NIJ = KH * KW                      # 40 kernel offsets
SCALE = 1.0 / D
NGRP = 14                          # phase-2 groups of <=3 offsets (3*13+1)
SHIFTS = (0, 1, 2, 3, 64, 65, 66, 67)

F8 = ml_dtypes.float8_e4m3
BF16 = ml_dtypes.bfloat16

_CACHE = {}


def _build_nc():
    nc = bass.Bass()
    z4_d = nc.declare_dram_parameter("z4", [KC, 4, ZF], DT.float8e4, isOutput=False)
    qp_d = nc.declare_dram_parameter("qp", [KC, 20, 2, PQ], DT.float8e4, isOutput=False)
    zt_d = nc.declare_dram_parameter(
        "zt", [128, 8, ZTC, KC], DT.float8e4, isOutput=False
    )
    mk_d = nc.declare_dram_parameter("mk", [128, T], DT.float32, isOutput=False)
    out_d = nc.declare_dram_parameter("out", [KC, NIJ * PQ], DT.bfloat16, isOutput=True)
    den_d = nc.declare_dram_parameter("den", [1, PQ], DT.float32, isOutput=True)

    from contextlib import ExitStack

    ctx = ExitStack()
    with ctx:
        z4_sb = ctx.enter_context(nc.sbuf_tensor([KC, 4, ZF], DT.float8e4))
        qp_sb = ctx.enter_context(nc.sbuf_tensor([KC, 20, 2, PQ], DT.float8e4))
        zt_sb = ctx.enter_context(nc.sbuf_tensor([128, 8, ZTC, KC], DT.float8e4))
        mk_sb = ctx.enter_context(nc.sbuf_tensor([128, T], DT.float32))
        e_sb = ctx.enter_context(nc.sbuf_tensor([128, T, PQ], DT.float32))
        eT_sb = ctx.enter_context(nc.sbuf_tensor([128, T, PQ], DT.float8e4))
        o_sb = ctx.enter_context(nc.sbuf_tensor([KC, NIJ * PQ], DT.bfloat16))
        den_sb = ctx.enter_context(nc.sbuf_tensor([1, PQ], DT.float32))
        wz = ctx.enter_context(nc.sbuf_tensor([128, 2, 512], DT.float8e4))
        ones8 = ctx.enter_context(nc.sbuf_tensor([128, 2], DT.float8e4))

        ps1 = [
            ctx.enter_context(nc.psum_tensor(f"ps1_{i}", [128, 512], DT.float32))
            for i in range(2)
        ]
        psD = ctx.enter_context(nc.psum_tensor("psD", [128, 512], DT.float32))
        ps2 = [
            ctx.enter_context(nc.psum_tensor(f"ps2_{i}", [128, 512], DT.float32))
            for i in range(5)
        ]
        # phase-2 group g -> bank (7-way rotation)
        BANKS = ps2 + [ps1[0], ps1[1]]

        s_g = ctx.enter_context(nc.semaphore("s_g"))
        s_z01 = ctx.enter_context(nc.semaphore("s_z01"))
        s_z23 = ctx.enter_context(nc.semaphore("s_z23"))
        s_qa = ctx.enter_context(nc.semaphore("s_qa"))
        s_qb = ctx.enter_context(nc.semaphore("s_qb"))
        s_zt = ctx.enter_context(nc.semaphore("s_zt"))
        s_mk = ctx.enter_context(nc.semaphore("s_mk"))
        s_p1 = ctx.enter_context(nc.semaphore("s_p1"))
        s_e = ctx.enter_context(nc.semaphore("s_e"))
        s_f = ctx.enter_context(nc.semaphore("s_f"))
        s_pd = ctx.enter_context(nc.semaphore("s_pd"))
        s_dc = ctx.enter_context(nc.semaphore("s_dc"))
        s_p2 = ctx.enter_context(nc.semaphore("s_p2"))
        s_cpA = ctx.enter_context(nc.semaphore("s_cpA"))
        s_cpV = ctx.enter_context(nc.semaphore("s_cpV"))
        s_o = ctx.enter_context(nc.semaphore("s_o"))

        # phase-2 groups: (group, [ij list]); ij = i*4 + j
        GROUPS = [list(range(3 * g, min(3 * g + 3, NIJ))) for g in range(NGRP)]
        # copy engine per group: even -> ACT, odd -> DVE
        def copy_wait(g):
            # semaphore + value that signals group g's psum copy is done
            return (s_cpA, g // 2 + 1) if g % 2 == 0 else (s_cpV, g // 2 + 1)

        with nc.Block() as block:

            @block.gpsimd
            def _(g):
                g.memset(wz[:], 0.0).then_inc(s_g, 1)
                g.memset(ones8[:], 1.0).then_inc(s_g, 1)

            @block.sync
            def _(sync):
                sync.dma_start(z4_sb[:, 0:2, :], z4_d[:, 0:2, :]).then_inc(s_z01, 16)
                sync.dma_start(qp_sb[:, 0:10, :, :], qp_d[:, 0:10, :, :]).then_inc(
                    s_qa, 16
                )
                sync.dma_start(z4_sb[:, 2:4, :], z4_d[:, 2:4, :]).then_inc(s_z23, 16)
                sync.dma_start(qp_sb[:, 10:20, :, :], qp_d[:, 10:20, :, :]).then_inc(
                    s_qb, 16
                )
                sync.dma_start(mk_sb[:], mk_d[:]).then_inc(s_mk, 16)
                sync.dma_start(zt_sb[:], zt_d[:]).then_inc(s_zt, 16)
                sync.wait_ge(s_dc, 1)
                sync.dma_start(den_d[:], den_sb[:]).then_inc(s_o, 16)
                # out pieces after groups {0-3, 4-7, 8-11, 12-13}
                PIECES = [(0, 4), (4, 8), (8, 12), (12, 14)]
                for g0, g1 in PIECES:
                    na = sum(1 for g in range(g1) if g % 2 == 0)
                    nv = g1 - na
                    sync.wait_ge(s_cpA, na)
                    sync.wait_ge(s_cpV, nv)
                    c0 = 480 * g0
                    c1 = min(480 * g1, NIJ * PQ)
                    sync.dma_start(out_d[:, c0:c1], o_sb[:, c0:c1]).then_inc(s_o, 16)
                sync.wait_ge(s_o, 80)

            @block.tensor
            def _(pe):
                # HAM warmup on the zeroed fp8 tile while input DMAs land
                pe.wait_ge(s_g, 1)
                for w_ in range(10):
                    nc.tensor.matmul(
                        psD[0:128, 0:512],
                        wz[:, :, 0:128],
                        wz[:, :, :],
                        start=(w_ == 0),
                        stop=(w_ == 9),
                        perf_mode=PM,
                    )
                # phase 1: 6 chunks, 20 DoubleRow matmuls each
                for t in range(T):
                    ps = ps1[t % 2]
                    if t >= 2:
                        pe.wait_ge(s_e, t - 1)  # bank drained by exp t-2
                    for pi in range(20):
                        jh, i_ = pi // 10, pi % 10
                        if t == 0 and pi == 0:
                            pe.wait_ge(s_z01, 16)
                            pe.wait_ge(s_qa, 16)
                        if t == 0 and pi == 10:
                            pe.wait_ge(s_z23, 16)
                            pe.wait_ge(s_qb, 16)
                        base = i_ * W + t * 128
                        mm = nc.tensor.matmul(
                            ps[0:128, 0:PQ],
                            z4_sb[:, 2 * jh : 2 * jh + 2, base : base + 128],
                            qp_sb[:, pi, :, :],
                            start=(pi == 0),
                            stop=(pi == 19),
                            perf_mode=PM,
                        )
                    mm.then_inc(s_p1, 1)
                # denominator: ones.T @ f over the 6 chunks (plain fp8)
                pe.wait_ge(s_f, T)
                pe.wait_ge(s_g, 2)
                for t in range(T):
                    mm = nc.tensor.matmul(
                        psD[0:1, 0:PQ],
                        ones8[:, 0:1],
                        eT_sb[:, t, :],
                        start=(t == 0),
                        stop=(t == T - 1),
                    )
                mm.then_inc(s_pd, 1)
                # phase 2: out3[c, ij, q] += zt.T @ f, 3 DoubleRow per ij
                pe.wait_ge(s_zt, 16)
                for g, ijs in enumerate(GROUPS):
                    if g >= 7:
                        sem, val = copy_wait(g - 7)
                        pe.wait_ge(sem, val)
                    bank = BANKS[g % 7]
                    for r, ij in enumerate(ijs):
                        i_, j_ = ij // KW, ij % KW
                        s_idx = (i_ % 2) * 4 + j_
                        di = i_ // 2
                        for tp in range(3):
                            mm = nc.tensor.matmul(
                                bank[0:128, r * PQ : (r + 1) * PQ],
                                zt_sb[:, s_idx, 2 * tp + di : 2 * tp + di + 2, :],
                                eT_sb[:, 2 * tp : 2 * tp + 2, :],
                                start=(tp == 0),
                                stop=(tp == 2),
                                perf_mode=PM,
                            )
                    mm.then_inc(s_p2, 1)

            @block.scalar
            def _(act):
                for t in range(T):
                    act.wait_ge(s_p1, t + 1)
                    nc.scalar.activation(
                        e_sb[:, t, :], ps1[t % 2][0:128, 0:PQ], AF.Exp, scale=SCALE
                    ).then_inc(s_e, 1)
                act.wait_ge(s_pd, 1)
                nc.scalar.activation(
                    den_sb[:], psD[0:1, 0:PQ], AF.Copy
                ).then_inc(s_dc, 1)
                for g in range(0, NGRP, 2):
                    act.wait_ge(s_p2, g + 1)
                    ncol = len(GROUPS[g]) * PQ
                    nc.scalar.activation(
                        o_sb[:, 480 * g : 480 * g + ncol],
                        BANKS[g % 7][0:128, 0:ncol],
                        AF.Copy,
                    ).then_inc(s_cpA, 1)

            @block.vector
            def _(dve):
                for t in range(T):
                    if t == 0:
                        dve.wait_ge(s_mk, 16)
                    dve.wait_ge(s_e, t + 1)
                    nc.vector.tensor_scalar(
                        eT_sb[:, t, :],
                        e_sb[:, t, :],
                        mk_sb[:, t : t + 1],
                        mk_sb[:, t : t + 1],
                        ALU.mult,
                        ALU.subtract,
                    ).then_inc(s_f, 1)
                for g in range(1, NGRP, 2):
                    dve.wait_ge(s_p2, g + 1)
                    ncol = len(GROUPS[g]) * PQ
                    nc.vector.tensor_copy(
                        o_sb[:, 480 * g : 480 * g + ncol],
                        BANKS[g % 7][0:128, 0:ncol],
                    ).then_inc(s_cpV, 1)

    return nc


def _host_prep(z1_hat, z2):
    z1 = np.asarray(z1_hat, dtype=np.float32)[0]  # [128, 100, 64]
    z2a = np.asarray(z2, dtype=np.float32)[0]

    # q patches -> paired lhs-side layout qp [128, 20, 2, 160] fp8
    q = z1.reshape(KC, NH, KH, NW, KW).transpose(1, 3, 0, 2, 4).reshape(PQ, D)
    qT3 = q.reshape(PQ, KC, KH * KW).transpose(1, 2, 0)  # [128, 40, 160]
    qp = (
        qT3.reshape(KC, NH, 2, 2, PQ)
        .transpose(0, 2, 1, 3, 4)
        .reshape(KC, 20, 2, PQ)
        .astype(F8)
    )
    qp = np.ascontiguousarray(qp)

    # padded z2 (rows 100..111 zero), fp8, flattened
    z_pad = np.zeros((KC, 112, W), dtype=np.float32)
    z_pad[:, :H] = z2a
    z8 = z_pad.astype(F8).reshape(KC, 112 * W)
    z8T = np.ascontiguousarray(z8.T)  # [7168, 128]

    in_maps = []
    for core in range(NCORES):
        base = HPC * core * W
        z4 = np.ascontiguousarray(
            np.stack([z8[:, base + j : base + j + ZF] for j in range(4)], axis=1)
        )
        zt = np.ascontiguousarray(
            np.stack(
                [
                    z8T[base + s : base + s + ZTC * 128]
                    .reshape(ZTC, 128, KC)
                    .transpose(1, 0, 2)
                    for s in SHIFTS
                ],
                axis=1,
            )
        )
        pos = np.arange(PKC)
        valid = (pos % W < WK) & (HPC * core + pos // W < HK)
        mk = np.ascontiguousarray(
            valid.astype(np.float32).reshape(T, 128).T
        )  # [128, 6]
        in_maps.append({"z4": z4, "qp": qp, "zt": zt, "mk": mk})

    # exact colsum term: colsum[c, i, j] = sum_{h<91, w<61} z2[c, h+i, w+j]
    ii = np.zeros((KC, H + 1, W + 1), dtype=np.float64)
    ii[:, 1:, 1:] = np.cumsum(np.cumsum(z2a, axis=1), axis=2)
    colsum = np.empty((KC, KH, KW), dtype=np.float64)
    for i in range(KH):
        for j in range(KW):
            colsum[:, i, j] = (
                ii[:, i + HK, j + WK] - ii[:, i, j + WK] - ii[:, i + HK, j] + ii[:, i, j]
            )
    return in_maps, colsum.reshape(KC, NIJ)


def kernel(z1_hat, z2):
    from concourse.bass_utils import run_bass_kernel_spmd

    in_maps, colsum = _host_prep(z1_hat, z2)
    if "nc" not in _CACHE:
        _CACHE["nc"] = _build_nc()
    nc = _CACHE["nc"]
    res = run_bass_kernel_spmd(nc, in_maps, list(range(NCORES)))
    num = np.broadcast_to(colsum[:, :, None], (KC, NIJ, PQ)).astype(np.float64).copy()
    den = np.full((PQ,), float(HK * WK), dtype=np.float64)
    for r in res.results:
        num += r["out"].astype(np.float64).reshape(KC, NIJ, PQ)
        den += r["den"].astype(np.float64)[0]
    out4 = num / den[None, None, :]
    # fold back: [c, (kh,kw), (nh,nw)] -> [1, 128, 100, 64]
    out4 = out4.reshape(KC, KH, KW, NH, NW).transpose(0, 3, 1, 4, 2)
    return np.ascontiguousarray(out4.reshape(1, KC, H, W).astype(np.float32))
